# revision 18
# baseline (speedup 1.0000x reference)
"""Trainium2 Bass kernel for Conf-MPU loss (nn_Conf_MPULoss) — v10.

Host side: rows sorted by label t into 5 class groups, split evenly across 8
cores, each per-core class segment padded to S = 128*R rows with sentinel rows
(exact in bf16). x ships as bf16 PLANAR per segment: [P, 6 planes, R]. For
segment c<4 the plane order is [j0, j1, j2, x4, x_c, -x_c] (j = non-c classes
ascending, so plane 3 is always the negative-class logit and plane 5 is
pre-negated x_c: one contiguous ScalarE exp also yields 1/e_c). Segment 4 uses
natural order, plane 5 not transferred. risk1-risk3 needs only sum(x4-x_c)
over rows with t=c, computed exactly on the host (f64) during packing.

Device per class segment c (planes as [P, R] bf16 slices of E = exp(X)):
    exp  : chunked ScalarE instrs -> e0..e3, e_c, 1/e_c
    a    = [e0+e2 | e1+e3]       one 2R-wide DVE 2x TT
    zp   = a0 + a1 ; z = zp + e_c ; lnz = ln(z)     (ScalarE ln)
    m    = (e_c > zp)  (== p_c > 1/2)     ze5 = z * (1/e_c)   (pre-ln)
    d4   = lnz - x4 ; u = d4 * ze5 ; g = m * u                (post-ln)
    c==4: max-tree over e-planes, mn = (2*max <= z) STT, g = mn*d4
ALL vector work runs on DVE in bf16 2x mode — the Pool engine is kept idle
(concurrent Pool tensor ops thrash SBUF ports and triple DVE latencies) and
SWDGE DMA is avoided (descriptor rings live in SBUF). Per-class sums
(den=sum m, num=sum g, li=sum g4) are colsum-matmuls with host-provided
one-hot ones-columns on the otherwise-idle PE into PSUM rows, extracted by
two DVE tensor_scalar accums -> [9,1] f32 -> two tiny DMAs out. Host
all-reduces the 9-vector across cores and does the final combination.

All input DMA rides the sync HWDGE ring in strict compute order, chunked so
each exp's data (+~2us completion-receipt lag) lands just before ScalarE
needs it; ScalarE (29R exp + 5R ln = the binding engine at ~26.5us busy)
streams with <1.5us total stall. Segment 3 is emitted last with its ln and
product chain split 15/16 + 1/16 so only a ~50-column chain, one matmul and
a 1-row extract trail the final ln. Exec ~45.5us: ~10.9 fixed head (NEFF
launch, instruction paging, DMA ramp), ~27.5 ACT stream, ~7 tail (chain +
DMA receipt + Tile sem-clear epilogue).
"""

import ml_dtypes
import numpy as np

import concourse.bacc as bacc
import concourse.mybir as mybir
import concourse.tile as tile
from concourse import bass_utils

F32 = mybir.dt.float32
BF16 = mybir.dt.bfloat16
Alu = mybir.AluOpType
Act = mybir.ActivationFunctionType

P = 128
NCLS = 5
N_CORES = 8
# stat rows: den c -> c (c<4), num c -> 4+c, li -> 8
NSTAT = 9
PSW = 512  # psum bank free width (f32)

_PROGRAM_CACHE: dict[int, tuple] = {}


def _restrict_act_tables(arch: str):
    """Confine Exp/Ln to the natural_log_exp_and_others set so the act-table
    pass emits a single ACT_TABLE_LOAD instead of thrashing between the
    exp_and_others and natural_log sets (~1.3us per load)."""
    from concourse import hw_specs

    tables = hw_specs.get_activation_tables(arch)
    if "natural_log_exp_and_others" not in tables:
        return
    for name, funcs in tables.items():
        if name != "natural_log_exp_and_others":
            funcs.discard(Act.Exp)
            funcs.discard(Act.Ln)


def _build_program(R: int):
    """Build + compile the per-core Bass program for segment length S=128*R."""
    nc = bacc.Bacc("TRN2", debug=False, num_devices=N_CORES)
    _restrict_act_tables(nc.m.arch)
    x_d = nc.dram_tensor("x", [NCLS, P, 6 * R], BF16, kind="ExternalInput").ap()
    w_d = nc.dram_tensor("w", [P, NSTAT * NSTAT + 1], BF16, kind="ExternalInput").ap()
    st_d = nc.dram_tensor("stats", [NSTAT, 1], F32, kind="ExternalOutput").ap()

    with tile.TileContext(nc) as tc:
        with (
            tc.tile_pool(name="io", bufs=1) as iop,
            tc.tile_pool(name="ep", bufs=1) as epool,
            tc.tile_pool(name="wk", bufs=1) as wp,
            tc.tile_pool(name="st", bufs=1) as sp,
            tc.tile_pool(name="ps", bufs=1, space="PSUM") as pp,
        ):
            # per-stat one-hot weight columns (host-filled): W_s =
            # wall[:, 9s:9s+9] has ones only in column s, so matmul adds
            # colsums into psum row s only; last column = plain ones (num3).
            wall = sp.tile([P, NSTAT * NSTAT + 1], BF16)
            psum = pp.tile([NSTAT, PSW], F32)
            psum2 = pp.tile([1, PSW], F32)
            ext = sp.tile([NSTAT, PSW], F32)
            ext2 = sp.tile([1, PSW], F32)
            stats = sp.tile([NSTAT, 1], F32)
            stats2 = sp.tile([1, 1], F32)
            wones = wall[:, 0 : NSTAT * NSTAT]
            wcol = wall[:, NSTAT * NSTAT : NSTAT * NSTAT + 1]

            X = {}
            E = {}
            seg = {}

            def alloc(c):
                n = 6 if c < 4 else 5
                X[c] = iop.tile([P, n * R], BF16, tag=f"x{c}", name=f"x{c}")
                E[c] = epool.tile([P, n * R], BF16, tag=f"e{c}", name=f"e{c}")
                s = {}
                names = ("a", "zp", "z", "lnz", "d4", "m", "g")
                if c < 4:
                    names += ("ze5", "u")
                else:
                    names += ("mm", "m3", "mx")
                for t in names:
                    w = 2 * R if t in ("a", "mm") else R
                    s[t] = wp.tile([P, w], BF16, tag=f"{t}_{c}", name=f"{t}_{c}")
                if c == 3:
                    # second-half chunk gets its own tiles so the tail chunks
                    # have zero tile overlap (no cross-engine WAR serialization)
                    for t in ("lnz", "d4", "u", "g"):
                        s[t + "b"] = wp.tile([P, R], BF16, tag=f"{t}b", name=f"{t}b")
                seg[c] = s

            for c in range(NCLS):
                alloc(c)

            def dma_in(c, jlo, jhi, eng):
                eng.dma_start(
                    out=X[c][:, jlo * R : jhi * R], in_=x_d[c][:, jlo * R : jhi * R]
                )

            def exp(c, jlo, jhi):
                nc.scalar.activation(
                    E[c][:, jlo * R : jhi * R], X[c][:, jlo * R : jhi * R], Act.Exp
                )

            def ln(c, lo=0, hi=None, b=False):
                hi = R if hi is None else hi
                s = seg[c]
                dst = s["lnzb"] if b else s["lnz"]
                off = lo if b else 0
                nc.scalar.activation(
                    dst[:, lo - off : hi - off], s["z"][:, lo:hi], Act.Ln
                )

            def pair(c):
                # a = [e0+e2 | e1+e3] as one 2R-wide 2x TT (planes 0:4)
                s = seg[c]
                nc.vector.tensor_tensor(
                    out=s["a"], in0=E[c][:, 0 : 2 * R], in1=E[c][:, 2 * R : 4 * R],
                    op=Alu.add,
                )

            def max_pair(c):
                s = seg[c]
                nc.vector.tensor_tensor(
                    out=s["mm"], in0=E[c][:, 0 : 2 * R], in1=E[c][:, 2 * R : 4 * R],
                    op=Alu.max,
                )

            def zsum(c):
                # zp = pa1 + pa2 (sum of the 4 non-c exps; needs planes 0:4)
                s = seg[c]
                nc.vector.tensor_tensor(
                    out=s["zp"], in0=s["a"][:, 0:R], in1=s["a"][:, R : 2 * R], op=Alu.add
                )

            def zonly(c):
                # z = zp + e_c (needs plane 4 only) — the ln-critical op
                s = seg[c]
                nc.vector.tensor_tensor(
                    out=s["z"], in0=s["zp"], in1=E[c][:, 4 * R : 5 * R], op=Alu.add
                )

            def zrest(c):
                # m = e_c > zp ; ze5 = z/e_c (needs plane 5)
                s = seg[c]
                nc.vector.tensor_tensor(
                    out=s["m"], in0=E[c][:, 4 * R : 5 * R], in1=s["zp"], op=Alu.is_gt
                )
                nc.vector.tensor_tensor(
                    out=s["ze5"], in0=s["z"], in1=E[c][:, 5 * R : 6 * R],
                    op=Alu.mult,
                )

            def zfin(c):
                zonly(c)
                if c < 4:
                    zrest(c)

            def maxtree4a():
                s = seg[4]
                nc.vector.tensor_tensor(
                    out=s["m3"], in0=s["mm"][:, 0:R], in1=s["mm"][:, R : 2 * R],
                    op=Alu.max,
                )

            def maxtree4b():
                s = seg[4]
                nc.vector.tensor_tensor(
                    out=s["mx"], in0=s["m3"], in1=E[4][:, 4 * R : 5 * R], op=Alu.max
                )

            def grp(c, lo=0, hi=None, b=False):
                # post-ln product chain; b=True -> seg3 second-half tiles
                # (relative 0-based slices) so the tail has no tile overlap
                hi = R if hi is None else hi
                s = seg[c]
                off = lo if b else 0
                sfx = "b" if b else ""

                def wb(t):  # chunk-local output tiles
                    return s[t + sfx][:, lo - off : hi - off]

                def wa(t):  # full-R shared input tiles (absolute slice)
                    return s[t][:, lo:hi]

                if c < 4:
                    # d4 = lnz - x4 (plane 3); u = d4 * z/e_c; g = m * u
                    nc.vector.tensor_tensor(
                        out=wb("d4"), in0=wb("lnz"),
                        in1=X[c][:, 3 * R + lo : 3 * R + hi], op=Alu.subtract,
                    )
                    nc.vector.tensor_tensor(
                        out=wb("u"), in0=wb("d4"), in1=wa("ze5"), op=Alu.mult
                    )
                    nc.vector.tensor_tensor(
                        out=wb("g"), in0=wa("m"), in1=wb("u"), op=Alu.mult
                    )
                else:
                    # d4 = lnz - x4 (plane 4); mn = (2*mx <= z); g = mn * d4
                    nc.vector.tensor_tensor(
                        out=wa("d4"), in0=wa("lnz"),
                        in1=X[c][:, 4 * R + lo : 4 * R + hi], op=Alu.subtract,
                    )
                    nc.vector.scalar_tensor_tensor(
                        out=wa("m"), in0=wa("mx"), scalar=2.0, in1=wa("z"),
                        op0=Alu.mult, op1=Alu.is_le,
                    )
                    nc.vector.tensor_tensor(
                        out=wa("g"), in0=wa("m"), in1=wa("d4"), op=Alu.mult
                    )

            def colsum(row, src, lo, hi, first, last):
                # psum[row] += per-column sums of src[:, lo:hi] via a one-hot
                # ones-column matmul. All stats share one accumulation group
                # on the [NSTAT, PSW] region; `first`/`last` only for the very
                # first/last matmul overall.
                cap = 256 if row is None else PSW
                chunks = []
                a = lo
                while a < hi:
                    b = min(a + cap, hi)
                    chunks.append((a, b))
                    a = b
                for i, (a, b) in enumerate(chunks):
                    if row is None:  # single-row bank (num3)
                        o, w = psum2[:, 0 : b - a], wcol
                    else:
                        o, w = psum[:, 0 : b - a], wones[:, NSTAT * row : NSTAT * row + NSTAT]
                    nc.tensor.matmul(
                        out=o,
                        lhsT=w,
                        rhs=src[:, a:b],
                        start=(first and i == 0),
                        stop=(last and i == len(chunks) - 1),
                        skip_group_check=True,
                    )

            h = R // 2
            q3 = (15 * R // 16 + 1) // 2 * 2  # seg3 split point (even)
            # DMA: seg4's first chunk rides the scalar HWDGE ring (free slot
            # before any activation work) while the sync ring starts on the
            # rest — both rings ramp in parallel so exp can start ~1.5us
            # earlier. Segments 1-3 as single transfers (DMA runs well ahead).
            dma_in(4, 0, 2, nc.sync)
            dma_in(4, 2, 4, nc.sync)
            dma_in(4, 4, 5, nc.sync)
            dma_in(0, 0, 4, nc.sync)
            dma_in(0, 4, 6, nc.sync)
            dma_in(1, 0, 4, nc.sync)
            dma_in(1, 4, 6, nc.sync)
            nc.sync.dma_start(out=wall, in_=w_d)
            dma_in(2, 0, 6, nc.sync)
            dma_in(3, 0, 4, nc.sync)
            dma_in(3, 4, 6, nc.sync)

            # ---- software-pipelined emission ----
            # stat rows: den c -> c, num c -> 4+c (c<3), li -> 7, num3 -> psum2
            exp(4, 0, 2)
            exp(4, 2, 4)
            pair(4)
            max_pair(4)
            exp(4, 4, 5)
            zsum(4)
            maxtree4a()
            zfin(4)
            maxtree4b()
            exp(0, 0, 4)
            ln(4)
            pair(0)
            zsum(0)
            grp(4)
            colsum(7, seg[4]["g"], 0, R, True, False)  # li (opens psum group)
            exp(0, 4, 6)
            zfin(0)
            exp(1, 0, 4)
            ln(0)
            colsum(0, seg[0]["m"], 0, R, False, False)  # den0
            pair(1)
            zsum(1)
            grp(0)
            colsum(4, seg[0]["g"], 0, R, False, False)  # num0
            exp(1, 4, 6)
            zfin(1)
            exp(2, 0, 6)
            ln(1)
            colsum(1, seg[1]["m"], 0, R, False, False)
            pair(2)
            zsum(2)
            zfin(2)
            grp(1)
            colsum(5, seg[1]["g"], 0, R, False, False)
            exp(3, 0, 4)
            ln(2)
            colsum(2, seg[2]["m"], 0, R, False, False)
            pair(3)
            zsum(3)
            grp(2)
            colsum(6, seg[2]["g"], 0, R, False, False)
            exp(3, 4, 5)
            zonly(3)
            exp(3, 5, 6)
            ln(3, 0, q3)
            zrest(3)
            colsum(3, seg[3]["m"], 0, R, False, True)  # den3 closes psum group
            grp(3, 0, q3)
            ln(3, q3, R, b=True)
            # main extract: psum rows 0..7 (den0-3, num0-2, li) while the
            # seg3 tail still runs
            nc.vector.tensor_scalar(
                out=ext[0:8, :],
                in0=psum[0:8, :],
                scalar1=1.0,
                scalar2=0.0,
                op0=Alu.mult,
                op1=Alu.add,
                accum_out=stats[0:8, :],
            )
            nc.sync.dma_start(out=st_d[0:8], in_=stats[0:8, :])
            colsum(None, seg[3]["g"], 0, q3, True, False)  # num3 -> psum2
            grp(3, q3, R, b=True)
            colsum(None, seg[3]["gb"], 0, R - q3, False, True)
            nc.vector.tensor_scalar(
                out=ext2[:, 0:256],
                in0=psum2[:, 0:256],
                scalar1=1.0,
                scalar2=0.0,
                op0=Alu.mult,
                op1=Alu.add,
                accum_out=stats2,
            )
            nc.sync.dma_start(out=st_d[8:9], in_=stats2)
    nc.compile()
    return nc


def _get_program(R: int):
    if R not in _PROGRAM_CACHE:
        _PROGRAM_CACHE[R] = _build_program(R)
    return _PROGRAM_CACHE[R]


def _prepare_inputs(x: np.ndarray, t: np.ndarray):
    """Sort rows by class, shard across cores, pad segments, pack planar bf16
    with per-segment plane permutation + negated-label plane. Also computes
    the exact host-side per-class sum(x4 - xc) (risk1-risk3 accumulator).
    Returns (in_maps, counts, sd, R)."""
    N = x.shape[0]
    t64 = t.astype(np.int64, copy=False)
    counts = np.bincount(t64, minlength=NCLS).astype(np.int64)

    n_ck = np.zeros((NCLS, N_CORES), dtype=np.int64)
    for c in range(NCLS):
        q, r = divmod(int(counts[c]), N_CORES)
        n_ck[c] = q
        n_ck[c, :r] += 1

    R = int(max(8, -(-int(n_ck.max()) // P)))
    R = (R + 1) // 2 * 2  # keep it even
    S = P * R

    order = np.argsort(t64, kind="stable")
    xs = np.ascontiguousarray(x[order], dtype=np.float32)
    starts = np.concatenate([[0], np.cumsum(counts)])

    # host-exact sum(x4 - xc) per positive class
    sd = np.zeros(4, dtype=np.float64)
    for c in range(4):
        blk = xs[int(starts[c]) : int(starts[c + 1])]
        sd[c] = blk[:, 4].astype(np.float64).sum() - blk[:, c].astype(np.float64).sum()

    # planar layout per (core, segment): [P, 6 planes, R]
    xcores = np.empty((N_CORES, NCLS, P, 6, R), dtype=np.float32)
    for c in range(NCLS):
        if c < 4:
            cols = [j for j in range(5) if j != c] + [c]
            padv = np.array([-10.0] * 3 + [10.0, -10.0, 10.0], dtype=np.float32)
        else:
            cols = [0, 1, 2, 3, 4]
            padv = np.array([-10.0] * 4 + [10.0, 0.0], dtype=np.float32)
        off = int(starts[c])
        for k in range(N_CORES):
            n = int(n_ck[c, k])
            blk = np.empty((S, 6), dtype=np.float32)
            if n:
                blk[:n, :5] = xs[off : off + n][:, cols]
                blk[:n, 5] = -blk[:n, 4] if c < 4 else 0.0
            blk[n:] = padv
            xcores[k, c] = blk.reshape(P, R, 6).transpose(0, 2, 1)
            off += n

    xb = xcores.reshape(N_CORES, NCLS, P, 6 * R).astype(ml_dtypes.bfloat16)
    w = np.zeros((P, NSTAT * NSTAT + 1), dtype=ml_dtypes.bfloat16)
    for s in range(NSTAT):
        w[:, NSTAT * s + s] = 1.0
    w[:, NSTAT * NSTAT] = 1.0
    in_maps = [{"x": xb[k], "w": w} for k in range(N_CORES)]
    return in_maps, counts, sd, R


def _combine(stats_list, counts, sd, N):
    """Host all-reduce of the per-class accumulators + final scalar combination."""
    st = np.zeros(NSTAT, dtype=np.float64)
    for s in stats_list:
        st += s.astype(np.float64).reshape(-1)

    counts = counts.astype(np.float64)
    r13 = 0.0  # risk1 - risk3
    r2 = 0.0
    for c in range(4):
        den = st[c]
        num = st[4 + c] if c < 3 else st[8]
        prior = counts[c] / N
        r13 += prior * sd[c] / max(1.0, counts[c])
        r2 += prior * num / max(den, 1.0)
    r4 = st[7] / max(1.0, counts[4])

    pos = 4.0 * (r13 + r2)
    if pos < 0.0:
        pos = 0.0
    return np.float32(pos + r4)


def run_device(in_maps, R, trace=False, **kw):
    nc = _get_program(R)
    res = bass_utils.run_bass_kernel_spmd(
        nc, in_maps, core_ids=list(range(N_CORES)), trace=trace, **kw
    )
    return res


def kernel(x: np.ndarray, t: np.ndarray) -> np.ndarray:
    x = np.asarray(x, dtype=np.float32)
    t = np.asarray(t)
    N = x.shape[0]
    in_maps, counts, sd, R = _prepare_inputs(x, t)
    res = run_device(in_maps, R)
    stats_list = [res.results[k]["stats"] for k in range(N_CORES)]
    return _combine(stats_list, counts, sd, N)
